# revision 3
# baseline (speedup 1.0000x reference)
"""KAN Fourier-linear kernel for 8 Trainium2 NeuronCores.

y[n,o] = sum_{i,g} C0[o,i,g]*cos(g*x[n,i]) + C1[o,i,g]*sin(g*x[n,i]) + bias[o]

Data-parallel over rows (4096 rows/core), fully on-chip features:
  - Odd harmonics g in {1,3,5,7,9,11,13,15} via ACT Sin with a shared
    round-based range reduction (Pool int32 round + DVE fused
    multiply-sub; Sin args stay in [-5pi/4, 5pi/4]).
  - Even harmonics via double-angle algebra on DVE/Pool/ACT:
      internal evens g=2,4,6,8 (chain inputs):
        sq = c_h^2 ; c_2h = 2*sq - 1 ; s_2h = (2*s_h)*c_h
      leaf evens g=10,12,14,16: the features ARE sq_h and s_h*c_h;
        the factor 2 / -1 offset are folded into weights/bias on host.
    This cuts ACT busy from ~122us (all-ACT baseline) to ~80us, so the
    PE (~110us for the K=4096 contraction) is the bottleneck.
  - Features and weights in fp16 (1 cycle/row on PE, ~10-bit mantissa;
    max rel err ~4e-3). MM_DTYPE env can select 'f32r' or 'bf16'.
  - Six superpasses (1024x3 + 512 + 256x2 columns) with alternating
    PSUM bank generations so one superpass's matmuls overlap the
    previous one's drains; the small final superpasses shrink the
    drain+DMA tail. PSUM accumulation uses start=True only on the
    first matmul touching each bank (lazy whole-bank zero, then
    overwrite-on-first-touch per element), which lets the startup
    pairs stream in 256-column pieces before a bank is "opened".
  - Startup: pairs 1 and 3 are produced piece-wise round-robin from
    256/512-column x chunks so the first matmul issues ~4us in; input
    DMAs are ordered by first use on the SP ring.
  - y.T tile = W.T @ F via PE, K=4096 accumulated in PSUM.
"""
import math
import os
import numpy as np
from contextlib import ExitStack

import concourse.bass as bass
import concourse.mybir as mybir
import concourse.tile as tile
from concourse import bacc
from concourse.bass_utils import run_bass_kernel_spmd

import ml_dtypes

N_CORES = 8
N_TOTAL = 32768
N_SHARD = N_TOTAL // N_CORES        # 4096 rows per core
INDIM = 128
OUTDIM = 256
GRID = 16
K_TOT = 2 * GRID * INDIM            # 4096
SP = 2                              # superpasses per core
S = N_SHARD // SP                   # 2048 cols per superpass
CH = 512                            # matmul moving chunk / psum bank
TWO_PI = 2.0 * math.pi

FP32 = mybir.dt.float32
F32R = mybir.dt.float32r
BF16 = mybir.dt.bfloat16
FP16 = mybir.dt.float16
I32 = mybir.dt.int32

MM_DTYPE = os.environ.get("MM_DTYPE", "fp16")  # 'f32r' or 'bf16'

ODD = [1, 3, 5, 7, 9, 11, 13, 15]
# (src harmonic h, dst harmonic 2h, is_leaf)
DOUBLINGS = [(1, 2, False), (2, 4, False), (4, 8, False), (8, 16, True),
             (3, 6, False), (6, 12, True), (5, 10, True), (7, 14, True)]
LEAF_EVEN = {g for _, g, leaf in DOUBLINGS if leaf}
INTERNAL_EVEN = {g for _, g, leaf in DOUBLINGS if not leaf}
# harmonics whose (c, s) tiles feed further algebra
CHAIN_INPUT = {h for h, _, _ in DOUBLINGS}

# production order of harmonic pairs (chains as soon as inputs exist)
PAIR_ORDER = [1, 3, 2, 5, 4, 7, 6, 9, 8, 10, 11, 16, 12, 13, 14, 15]
SPLITS = [1024, 1024, 1024, 512, 256, 256]  # superpass column widths
CHWS = [512, 512, 512, 512, 256, 256]  # matmul chunk width per superpass
CHUNKED_PAIRS = {1, 3}              # pairs produced in CH-column chunks (sp 0)
FBUFS = 24                          # feature pool buffers

# engine assignment for the algebra ops (tuned against CoreSim)
#   sq_h: square of c_h         (8 ops: h in 1..8)
#   sc_h: s_h*c_h for leaves    (4 ops: h in 5..8)
#   ts_g: c_g = 2*sq-1          (4 ops: internal evens)
#   sd_g: s_g = (2 s)*c         (4 ops: internal evens)
SQ_ENGINE = {1: "dve", 2: "act", 3: "pool", 4: "act",
             5: "pool", 6: "dve", 7: "dve", 8: "pool"}
SC_ENGINE = {5: "dve", 6: "pool", 7: "pool", 8: "dve"}
TS_ENGINE = {2: "pool", 4: "dve", 6: "dve", 8: "pool"}
SD_ENGINE = {2: "dve", 4: "dve", 6: "dve", 8: "dve"}

def _g_consts(g: int):
    a = np.float32(g / TWO_PI)
    phat = np.float32(TWO_PI / g)
    m = 2.0 ** math.ceil(math.log2(0.960 * g + 0.14))
    c = np.float32(m + 0.125)
    b_s = np.float32(m * g * float(phat))      # == 2pi*m up to fp32
    b_c = np.float32(float(b_s) + math.pi / 2.0)
    return a, phat, c, b_s, b_c


def _kt(g: int, t: int) -> int:
    """weight block index; t=0 cos-side, t=1 sin-side."""
    return (g - 1) * 2 + t


_CACHED = {}


def _build(repeat: int = 1):
    key = ("nc", MM_DTYPE, repeat)
    if key in _CACHED:
        return _CACHED[key]
    nc = bacc.Bacc("TRN2", target_bir_lowering=False, debug=False,
                   num_devices=N_CORES)
    w_dt = {"f32r": F32R, "bf16": BF16, "fp16": FP16}[MM_DTYPE]
    f_dt = w_dt                         # dtype PE reads features in

    xt_d = nc.dram_tensor("xt", [INDIM, N_SHARD], FP32, kind="ExternalInput").ap()
    w_d = nc.dram_tensor("w", [INDIM, 32 * OUTDIM], w_dt, kind="ExternalInput").ap()
    bt_d = nc.dram_tensor("bt", [INDIM, 16], FP32, kind="ExternalInput").ap()
    bias_d = nc.dram_tensor("bias", [INDIM, 2], FP32, kind="ExternalInput").ap()
    yt_d = nc.dram_tensor("yt", [OUTDIM, N_SHARD], FP32, kind="ExternalOutput").ap()

    odd_idx = {g: i for i, g in enumerate(ODD)}
    dbl = {g: (h, g, leaf) for h, g, leaf in DOUBLINGS}

    with tile.TileContext(nc) as tc, ExitStack() as ctx:
        cpool = ctx.enter_context(tc.tile_pool(name="const", bufs=1))
        vpool = ctx.enter_context(tc.tile_pool(name="v", bufs=2))
        rpool = ctx.enter_context(tc.tile_pool(name="r", bufs=2))
        fpool = ctx.enter_context(tc.tile_pool(name="f", bufs=FBUFS))
        ypool = ctx.enter_context(tc.tile_pool(name="y", bufs=2))
        ppool = ctx.enter_context(tc.tile_pool(name="psum", bufs=1, space="PSUM"))

        xt = cpool.tile([INDIM, N_SHARD], FP32)
        wt = cpool.tile([INDIM, 32 * OUTDIM], w_dt)
        bt = cpool.tile([INDIM, 16], FP32)
        bias = cpool.tile([INDIM, 2], FP32)
        warm = cpool.tile([INDIM, 1], FP32)

        engines = {"dve": nc.vector, "pool": nc.gpsimd, "act": None}

        def emit_body():
            # preload the Sin activation table off the critical path
            nc.vector.memset(warm[:], 0.0)
            nc.scalar.activation(warm[:], warm[:],
                                 mybir.ActivationFunctionType.Sin)
            # all input DMAs on the SP ring, ordered by first use
            nc.sync.dma_start(xt[:, 0:256], xt_d[:, 0:256])
            nc.sync.dma_start(wt[:, 0:512], w_d[:, 0:512])
            nc.sync.dma_start(bt[:], bt_d[:])
            nc.sync.dma_start(xt[:, 256:CH], xt_d[:, 256:CH])
            nc.sync.dma_start(wt[:, 512:1536], w_d[:, 512:1536])
            nc.sync.dma_start(xt[:, CH:SPLITS[0]], xt_d[:, CH:SPLITS[0]])
            nc.sync.dma_start(bias[:], bias_d[:])
            nc.sync.dma_start(wt[:, 1536:4096], w_d[:, 1536:4096])
            nc.sync.dma_start(xt[:, SPLITS[0]:2 * SPLITS[0]],
                              xt_d[:, SPLITS[0]:2 * SPLITS[0]])
            nc.sync.dma_start(wt[:, 4096:8192], w_d[:, 4096:8192])
            nc.sync.dma_start(xt[:, 2 * SPLITS[0]:N_SHARD],
                              xt_d[:, 2 * SPLITS[0]:N_SHARD])

            col0 = 0
            for sp, S_sp in enumerate(SPLITS):
                emit_superpass(sp, col0, S_sp)
                col0 += S_sp

        def emit_superpass(sp, col0, S_sp):
            CHW = CHWS[sp]
            nch = S_sp // CHW
            xs = xt[:, col0:col0 + S_sp]
            psums = {}
            for oh in range(2):
                for chi in range(nch):
                    # padded to a full 2KB bank so matmul outputs never
                    # share a bank across chunks
                    psums[(oh, chi)] = ppool.tile(
                        [128, CHW], FP32, name=f"ps{oh}{chi}",
                        tag=f"p{sp % 2}{oh}{chi}", padded_shape=[128, CH])

            feats = {}
            n_mm = [0]
            started = set()

            def mm_one(kt, oh, chi, rhs, stop, bsl=None):
                # start=True only on the first matmul touching a bank: it
                # lazily zeroes the whole bank; later writes overwrite on
                # first touch per element, then accumulate
                lhsT = wt[:, kt * OUTDIM + oh * 128:
                          kt * OUTDIM + oh * 128 + 128]
                out = psums[(oh, chi)][:, bsl] if bsl is not None \
                    else psums[(oh, chi)][:]
                start = (oh, chi) not in started
                started.add((oh, chi))
                nc.tensor.matmul(out, lhsT, rhs, start=start, stop=stop)

            def emit_mm(g, t, ap):
                fi = n_mm[0]
                kt = _kt(g, t)
                for oh in range(2):
                    for chi in range(nch):
                        mm_one(kt, oh, chi,
                               ap[:, chi * CHW:(chi + 1) * CHW],
                               fi == 31)
                n_mm[0] += 1

            def feat_tile(name):
                return fpool.tile([INDIM, S_sp], f_dt, name=name, tag="f")

            def chain_tile(name):
                return fpool.tile([INDIM, S_sp], f_dt, name=name, tag="f")

            def _odd_piece(g, p0, p1, pi):
                """range-reduce + Sin + MMs for columns [p0,p1) of pair g"""
                a, phat, c, b_s, b_c = _g_consts(g)
                oi = odd_idx[g]
                ct, st = feats[(g, 0)], feats[(g, 1)]
                sl = slice(p0, p1)
                pw = p1 - p0
                v = vpool.tile([INDIM, pw], I32, name=f"vc{g}{pi}",
                               tag="vs", bufs=4, padded_shape=[INDIM, 512])
                nc.gpsimd.tensor_scalar(v[:, :pw], xs[:, sl], float(a),
                                        float(c), mybir.AluOpType.mult,
                                        mybir.AluOpType.add)
                r = rpool.tile([INDIM, pw], FP32, name=f"rc{g}{pi}",
                               tag="rs", bufs=4, padded_shape=[INDIM, 512])
                nc.vector.scalar_tensor_tensor(
                    r[:, :pw], v[:, :pw], float(-phat), xs[:, sl],
                    mybir.AluOpType.mult, mybir.AluOpType.add)
                nc.scalar.activation(ct[:, sl], r[:, :pw],
                                     mybir.ActivationFunctionType.Sin,
                                     bias=bt[:, 2 * oi:2 * oi + 1],
                                     scale=float(g))
                nc.scalar.activation(st[:, sl], r[:, :pw],
                                     mybir.ActivationFunctionType.Sin,
                                     bias=bt[:, 2 * oi + 1:2 * oi + 2],
                                     scale=float(g))
                chi = p0 // CHW
                bsl = slice(p0 - chi * CHW, p1 - chi * CHW)
                for oh in range(2):
                    mm_one(_kt(g, 0), oh, chi, ct[:, sl],
                           feats["fi", g, 0] == 31, bsl)
                for oh in range(2):
                    mm_one(_kt(g, 1), oh, chi, st[:, sl],
                           feats["fi", g, 1] == 31, bsl)

            def produce_odd_group_chunked(gs):
                """pairs produced piece-wise round-robin so ACT interleaves
                them and the PE never waits on one pair's full tile"""
                plans = []
                for i, g in enumerate(gs):
                    feats[(g, 0)] = chain_tile(f"c{g}")
                    feats[(g, 1)] = chain_tile(f"s{g}")
                    feats["fi", g, 0] = n_mm[0]
                    feats["fi", g, 1] = n_mm[0] + 1
                    n_mm[0] += 2
                    if CHW == 512:
                        p = [(0, 256), (256, 512)]
                    else:
                        p = [(0, CHW)]
                    p += [(c * CHW, (c + 1) * CHW) for c in range(1, nch)]
                    plans.append((g, p))
                # round-robin the pieces across the group
                maxp = max(len(p) for _, p in plans)
                for pi in range(maxp):
                    for g, p in plans:
                        if pi < len(p):
                            _odd_piece(g, p[pi][0], p[pi][1], pi)

            def produce_odd(g):
                a, phat, c, b_s, b_c = _g_consts(g)
                v = vpool.tile([INDIM, S_sp], I32, name=f"v{g}", tag="v")
                nc.gpsimd.tensor_scalar(v[:], xs, float(a), float(c),
                                        mybir.AluOpType.mult,
                                        mybir.AluOpType.add)
                r = rpool.tile([INDIM, S_sp], FP32, name=f"r{g}", tag="r")
                nc.vector.scalar_tensor_tensor(r[:], v[:], float(-phat), xs,
                                               mybir.AluOpType.mult,
                                               mybir.AluOpType.add)
                oi = odd_idx[g]
                ct = chain_tile(f"c{g}")
                st = chain_tile(f"s{g}")
                nc.scalar.activation(ct[:], r[:],
                                     mybir.ActivationFunctionType.Sin,
                                     bias=bt[:, 2 * oi:2 * oi + 1],
                                     scale=float(g))
                nc.scalar.activation(st[:], r[:],
                                     mybir.ActivationFunctionType.Sin,
                                     bias=bt[:, 2 * oi + 1:2 * oi + 2],
                                     scale=float(g))
                feats[(g, 0)] = ct
                feats[(g, 1)] = st
                emit_mm(g, 0, ct)
                emit_mm(g, 1, st)

            def tt(eng, out, i0, i1):
                if eng == "act":
                    assert i0 is i1
                    nc.scalar.activation(out[:], i0[:],
                                         mybir.ActivationFunctionType.Square)
                else:
                    engines[eng].tensor_tensor(out[:], i0[:], i1[:],
                                               mybir.AluOpType.mult)

            def produce_double(h, g, leaf):
                c_h = feats[(h, 0)]
                s_h = feats[(h, 1)]
                sq = fpool.tile([INDIM, S_sp], f_dt, name=f"sq{h}", tag="f")
                tt(SQ_ENGINE[h], sq, c_h, c_h)
                if leaf:
                    sc = feat_tile(f"sc{h}")
                    tt(SC_ENGINE[h], sc, s_h, c_h)
                    feats[(g, 0)] = sq
                    feats[(g, 1)] = sc
                    emit_mm(g, 0, sq)
                    emit_mm(g, 1, sc)
                else:
                    ct = chain_tile(f"c{g}")
                    st = chain_tile(f"s{g}")
                    engines[TS_ENGINE[g]].tensor_scalar(
                        ct[:], sq[:], 2.0, -1.0,
                        mybir.AluOpType.mult, mybir.AluOpType.add)
                    engines[SD_ENGINE[g]].scalar_tensor_tensor(
                        st[:], s_h[:], 2.0, c_h[:],
                        mybir.AluOpType.mult, mybir.AluOpType.mult)
                    feats[(g, 0)] = ct
                    feats[(g, 1)] = st
                    emit_mm(g, 0, ct)
                    emit_mm(g, 1, st)

            assert PAIR_ORDER[0] == 1
            grouped = [g for g in PAIR_ORDER if g in CHUNKED_PAIRS] \
                if sp == 0 else []
            if grouped:
                produce_odd_group_chunked(grouped)
            for g in PAIR_ORDER:
                if g in grouped:
                    continue
                elif g in odd_idx:
                    produce_odd(g)
                else:
                    produce_double(*dbl[g])

            # drain psum banks (ACT for oh=0, DVE for oh=1 in parallel),
            # stream the output out in half-superpass pieces
            ys = {}
            for oh in range(2):
                ys[oh] = ypool.tile([128, S_sp], FP32, name=f"y{oh}",
                                    tag=f"y{oh}")
            for chi in range(nch):
                sl = slice(chi * CHW, (chi + 1) * CHW)
                nc.scalar.activation(
                    ys[0][:, sl], psums[(0, chi)][:],
                    mybir.ActivationFunctionType.Identity,
                    bias=bias[:, 0:1])
                nc.vector.tensor_scalar(
                    ys[1][:, sl], psums[(1, chi)][:],
                    bias[:, 1:2], None, mybir.AluOpType.add)
                if CHW >= 512:
                    for oh in range(2):
                        nc.sync.dma_start(
                            yt_d[oh * 128:(oh + 1) * 128,
                                 col0 + chi * CHW:col0 + (chi + 1) * CHW],
                            ys[oh][:, sl])
            if CHW < 512:
                for oh in range(2):
                    nc.sync.dma_start(
                        yt_d[oh * 128:(oh + 1) * 128, col0:col0 + S_sp],
                        ys[oh][:])

        if repeat > 1:
            with tc.For_i(0, repeat):
                emit_body()
        else:
            emit_body()

    nc.compile()
    _CACHED[key] = nc
    return nc


def _prep_inputs(x: np.ndarray, fouriercoeffs: np.ndarray, bias: np.ndarray):
    xt = np.ascontiguousarray(x.astype(np.float32, copy=False).T)  # (128, 32768)
    W = fouriercoeffs.astype(np.float64)          # (2, 256, 128, 16)
    Wk = np.empty((32, INDIM, OUTDIM), np.float64)  # [kt, i, o]
    for g in range(1, GRID + 1):
        gi = g - 1
        scale = 2.0 if g in LEAF_EVEN else 1.0
        for t in range(2):
            Wk[_kt(g, t)] = scale * W[t][:, :, gi].T
    w_sb = np.ascontiguousarray(
        Wk.transpose(1, 0, 2).reshape(INDIM, 32 * OUTDIM))
    w_sb = w_sb.astype({"f32r": np.float32, "bf16": ml_dtypes.bfloat16,
                        "fp16": np.float16}[MM_DTYPE])
    # bias folding for leaf-even cos features: w*(2sq-1) = (2w)*sq - w
    bias_eff = bias.astype(np.float64).reshape(-1).copy()
    for g in LEAF_EVEN:
        bias_eff -= W[0][:, :, g - 1].sum(axis=1)
    bias_sb = np.ascontiguousarray(
        bias_eff.reshape(2, 128).T.astype(np.float32))  # (128, 2)
    # ACT bias table for odd harmonics
    bvals = np.empty(16, np.float32)
    for oi, g in enumerate(ODD):
        _, _, _, b_s, b_c = _g_consts(g)
        bvals[2 * oi] = b_c
        bvals[2 * oi + 1] = b_s
    bt = np.tile(bvals[None, :], (INDIM, 1)).astype(np.float32)
    return xt, w_sb, bt, bias_sb


def sim_inputs(x2, fouriercoeffs, bias):
    xt, w_sb, bt, bias_sb = _prep_inputs(x2, fouriercoeffs, bias)
    return {
        "xt": np.ascontiguousarray(xt[:, :N_SHARD]),
        "w": w_sb,
        "bt": bt,
        "bias": bias_sb,
    }


def kernel(x: np.ndarray, fouriercoeffs: np.ndarray, bias: np.ndarray,
           _trace: bool = False):
    x = np.asarray(x)
    fouriercoeffs = np.asarray(fouriercoeffs)
    bias = np.asarray(bias)
    orig_shape = x.shape
    x2 = x.reshape(-1, INDIM)
    assert x2.shape == (N_TOTAL, INDIM), x2.shape

    nc = _build()
    xt, w_sb, bt, bias_sb = _prep_inputs(x2, fouriercoeffs, bias)
    in_maps = []
    for c in range(N_CORES):
        in_maps.append({
            "xt": np.ascontiguousarray(xt[:, c * N_SHARD:(c + 1) * N_SHARD]),
            "w": w_sb,
            "bt": bt,
            "bias": bias_sb,
        })
    try:
        res = run_bass_kernel_spmd(nc, in_maps, list(range(N_CORES)),
                                   trace=_trace)
    except Exception:
        if not _trace:
            raise
        # tracing infrastructure unavailable in this environment
        res = run_bass_kernel_spmd(nc, in_maps, list(range(N_CORES)),
                                   trace=False)
    yt = np.concatenate([res.results[c]["yt"] for c in range(N_CORES)], axis=1)
    y = np.ascontiguousarray(yt.T).astype(np.float32)
    if _trace:
        kernel._last_result = res
    return y.reshape(*orig_shape[:-1], OUTDIM)
